# revision 1
# baseline (speedup 1.0000x reference)
"""Trainium2 Bass kernel v5 for nn_PoM_22986664968549 (sparse_attention).

Reference (B=4, N=4096, DIM=128, DE=512):
    s   = xq @ W_se.T + b_se
    g   = gelu(xq @ W_po.T + b_po)          (exact erf)
    h   = concat([g1, g2*g1])
    agg = einsum('bnd,bmn->bmd', h, mask) / (1e-7 + sum(mask, n))
    out = (sigmoid(s) * agg) @ W_ag.T + b_ag

Design (vs the fp32r baseline):
  * All matmul operands bf16 (PE 1 cyc/row at any free size; fp32r pays
    4 cyc/row below 256 free). PSUM accumulates fp32. rel-err budget 2e-2.
  * Phase 1 computes h TRANSPOSED (feature-major): lhsT = W_po chunk is
    stationary, and b_po becomes a per-partition ACT bias fused into the
    Gelu read -- eliminating the baseline's rank-1 bias matmuls (one per
    block, 100% overhead on phase-1 PE).
  * Feature-major gelu/product results are transposed to query-major via
    batched DMA-transpose (xbar 16x128 tiles, ~14ns/tile, bf16) -- DMA
    engines are otherwise idle mid-kernel.
  * Aggregation is RAW (no 1/count): per out-block, 4 chunk matmuls with
    rhs = unscaled triangular constant, then the block-offset sum is
    ACCUMULATED INTO THE SAME PSUM via a variable-K matmul over the
    block-sums table S (rows: [T_other(2) | own S in device order]), with
    rhs = ones. Causal prefix = contiguous S row slice [0 : 2+k].
  * sigmoid gate: DVE/Pool tensor_tensor psum x sigT -> bf16, split
    across both engines to balance.
  * Output projection is QUERY-major (lhsT = gated oT chunk), so the
    1/count scale and b_ag commute out of the f-contraction and are
    applied exactly on the HOST after gather.
  * Every core computes H only as needed: own half for the triangular
    part + the other half only to form T = sum of other-half block sums
    (still requires other-half gelu in local mode; the S-emit constants
    carry a per-core mask so the same SPMD program works on both
    half-assignments).

Sharding: 8 cores = 4 batches x 2 query-halves (as baseline: host
permutes rows so device blocks 16..31 are the core's own half).
"""

import os
import sys

import numpy as np

sys.path.insert(0, "/opt/trn_rl_repo")

from concourse import bacc, bass, mybir, tile
from concourse.bass_utils import run_bass_kernel_spmd

B, N, DIM, DE = 4, 4096, 128, 512
NBLK = N // 128          # 32 device key blocks
OBLK = 16                # out blocks per core (device blocks 16..31)
HALF = OBLK * 128        # 2048
F32 = mybir.dt.float32
BF16 = mybir.dt.bfloat16
AF = mybir.ActivationFunctionType
OP = mybir.AluOpType

# S_sb row map: rows 0,1 = other-half total (paired cols), rows 2..17 =
# own device blocks 16..31 in order. Causal prefix for out-block k is
# S_sb[0 : 2+k].
SROWS = 18


def build_nc(mode, reps=None, parts="all"):
    """reps: if set, wrap the whole per-call body (input DMA + compute +
    output DMA) in a hardware For_i loop executing it `reps` times, with a
    tiny serializing DMA chaining each iteration's output to the next
    iteration's input load. parts: bisect for benchmarking --
    all | p12 (phases 1+2+S) | p3 (blocks only) | dma | p1 | p1nt
    (p1 without transposes/S)."""
    assert mode in ("causal", "ones")
    nc = bacc.Bacc("TRN2", target_bir_lowering=False, debug=False,
                   num_devices=8)

    xqT_d = nc.dram_tensor("xqT", [128, N], BF16, kind="ExternalInput")
    wpoT_d = nc.dram_tensor("wpoT", [128, DE], BF16, kind="ExternalInput")
    wseT_d = nc.dram_tensor("wseT", [128, DE], BF16, kind="ExternalInput")
    wagT_d = nc.dram_tensor("wagT", [128, 4, 128], BF16, kind="ExternalInput")
    bpoP_d = nc.dram_tensor("bpoP", [128, 4], F32, kind="ExternalInput")
    bse_d = nc.dram_tensor("bse", [128, 4], F32, kind="ExternalInput")
    utri_d = nc.dram_tensor("utri", [128, 128], BF16, kind="ExternalInput")
    onzT_d = nc.dram_tensor("onzT", [128, 8], BF16, kind="ExternalInput")
    onesk_d = nc.dram_tensor("onesk", [32, 128], BF16, kind="ExternalInput")
    negk_d = nc.dram_tensor("negk", [32, 128], BF16, kind="ExternalInput")
    ident_d = nc.dram_tensor("ident", [128, 128], BF16, kind="ExternalInput")
    tmask_d = nc.dram_tensor("tmask", [128, 1], F32, kind="ExternalInput")
    out_d = nc.dram_tensor("outQ", [128, OBLK, 128], BF16,
                           kind="ExternalOutput")

    with tile.TileContext(nc) as tc:
        with (
            tc.tile_pool(name="consts", bufs=1) as cp,
            tc.tile_pool(name="big", bufs=1) as bp,
        ):
            xqT = cp.tile([128, N], BF16)
            wpoT = cp.tile([128, DE], BF16)
            wseT = cp.tile([128, DE], BF16)
            wagT = cp.tile([128, 4, 128], BF16)
            bpoP = cp.tile([128, 4], F32)
            bse = cp.tile([128, 4], F32)
            utri = cp.tile([128, 128], BF16)
            onzT = cp.tile([128, 8], BF16)
            onesk = cp.tile([32, 128], BF16)
            negk = cp.tile([32, 128], BF16)
            ident = cp.tile([128, 128], BF16)
            tmask = cp.tile([128, 1], F32)

            HT = [bp.tile([128, N], BF16, name=f"HT{c}") for c in range(4)]
            GTt = [bp.tile([128, N], BF16, name=f"GTt{c}") for c in range(2)]
            H = [bp.tile([128, NBLK, 128], BF16, name=f"Hc{c}") for c in range(4)]
            Tacc = bp.tile([128, 4, 2], F32)
            S_sb = bp.tile([32, DE], BF16)
            sigT = bp.tile([128, 4, HALF], BF16)
            outQ = bp.tile([128, OBLK, 128], BF16)

            for dst, src in [
                (wpoT, wpoT_d), (wseT, wseT_d), (wagT, wagT_d),
                (bpoP, bpoP_d), (bse, bse_d), (utri, utri_d),
                (onzT, onzT_d), (onesk, onesk_d), (negk, negk_d),
                (ident, ident_d), (tmask, tmask_d),
            ]:
                nc.sync.dma_start(dst[:], src[:])

            if parts != "all":
                # prefill tiles that skipped stages would have written
                for t in H + HT + GTt + [S_sb, sigT, outQ]:
                    nc.vector.memset(t[:], 0.0)
                nc.vector.memset(Tacc[:], 0.0)

            def body():
                emit_body(nc, tc, mode, xqT, xqT_d, wpoT, wseT, wagT, bpoP,
                          bse, utri, onzT, onesk, negk, ident, tmask, HT,
                          GTt, H, Tacc, S_sb, sigT, outQ, out_d,
                          serialize=reps is not None, parts=parts)

            if reps is None:
                body()
            else:
                with tc.For_i(0, reps):
                    body()

    nc.compile()
    return nc


def emit_body(nc, tc, mode, xqT, xqT_d, wpoT, wseT, wagT, bpoP, bse, utri,
              onzT, onesk, negk, ident, tmask, HT, GTt, H, Tacc, S_sb, sigT,
              outQ, out_d, serialize=False, parts="all"):
    do_p1 = parts in ("all", "p12", "p1", "p1nt")
    do_tr = parts in ("all", "p12", "p1")
    do_p2 = parts in ("all", "p12")
    do_p3 = parts in ("all", "p3")
    if True:
        if True:
            if parts != "nul":
                # own half (cols 2048+) is consumed first by phase 1 --
                # load it first so the first gelu isn't waiting on the
                # other half's chunks
                for ch in (4, 5, 6, 7, 0, 1, 2, 3):
                    sl = slice(ch * 512, (ch + 1) * 512)
                    nc.sync.dma_start(xqT[:, sl], xqT_d[:, sl])

            # ---- Phase 1: feature-major projection + gelu (+bias fused)
            # + po2 product + DMA-transpose to query-major + S emit.
            # Seg pairs of 1024 keys; own half (device cols 2048..4096)
            # first so phase-3 tri matmuls can start early.
            with (
                tc.tile_pool(name="hps", bufs=3, space="PSUM") as hp,
                tc.tile_pool(name="stps", bufs=1, space="PSUM") as stp,
            ):
                stT_ps = stp.tile([128, 4, 32], F32, name="stT_ps")
                if not do_p1:
                    pair_list_ = []
                elif not do_tr:
                    pair_list_ = "nt"
                else:
                    pair_list_ = None

                # stT_ps is one PSUM bank = one zero region: all S-emit
                # matmuls form a single accumulation group (first zeroes
                # the bank, last stops it).
                s_first = [True]

                def emit_S(j, colpair, sel, last):
                    for c in range(4):
                        nc.tensor.matmul(
                            stT_ps[:, c, colpair:colpair + 2],
                            H[c][:, j, :],
                            onzT[:, sel:sel + 2],
                            start=s_first[0], stop=(last and c == 3),
                        )
                        s_first[0] = False

                pair_list = [(2048, 4), (3072, 4), (0, 0), (1024, 0)]
                if pair_list_ == []:
                    pair_list = []
                for n0, _own in pair_list:
                    own = n0 >= 2048
                    slot = (n0 // 1024) % 2
                    for c in range(4):
                        ps = hp.tile([128, 2, 512], F32)
                        lhs = wpoT[:, c * 128:(c + 1) * 128]
                        nc.tensor.matmul(ps[:, 0, :], lhs,
                                         xqT[:, n0:n0 + 512],
                                         start=True, stop=True)
                        nc.tensor.matmul(ps[:, 1, :], lhs,
                                         xqT[:, n0 + 512:n0 + 1024],
                                         start=True, stop=True)
                        tgt = HT[c] if c < 2 else GTt[c - 2]
                        # other half: per-chunk totals ride the gelu /
                        # product ops as free-dim accumulator side-outputs
                        acc = (Tacc[:, c, slot:slot + 1]
                               if (not own and c < 2) else None)
                        nc.scalar.activation(
                            tgt[:, n0:n0 + 1024], ps[:, :, :], AF.Gelu,
                            bias=bpoP[:, c:c + 1], scale=1.0,
                            accum_out=acc,
                        )
                    for c in (2, 3):
                        if own:
                            nc.vector.tensor_tensor(
                                HT[c][:, n0:n0 + 1024],
                                GTt[c - 2][:, n0:n0 + 1024],
                                HT[c - 2][:, n0:n0 + 1024], op=OP.mult,
                            )
                        else:
                            nc.vector.scalar_tensor_tensor(
                                HT[c][:, n0:n0 + 1024],
                                GTt[c - 2][:, n0:n0 + 1024], 1.0,
                                HT[c - 2][:, n0:n0 + 1024],
                                op0=OP.mult, op1=OP.mult,
                                accum_out=Tacc[:, c, slot:slot + 1],
                            )
                    if not do_tr or not own:
                        continue
                    j0 = n0 // 128
                    for c in range(4):
                        nc.sync.dma_start_transpose(
                            H[c][:, j0:j0 + 8, :],
                            HT[c][:, n0:n0 + 1024],
                        )
                    for b in range(8):
                        j = j0 + b
                        k = j - 16
                        emit_S(j, 2 + (k // 2) * 2,
                               0 if k % 2 == 0 else 2, last=(k == 15))

                # ---- Phase 2: sigT = sigmoid(s)^T for own half ----
                for p in (range(2) if do_p2 else ()):
                    q0 = HALF + p * 1024
                    for c in range(4):
                        ps = hp.tile([128, 2, 512], F32)
                        lhs = wseT[:, c * 128:(c + 1) * 128]
                        nc.tensor.matmul(ps[:, 0, :], lhs,
                                         xqT[:, q0:q0 + 512],
                                         start=True, stop=True)
                        nc.tensor.matmul(ps[:, 1, :], lhs,
                                         xqT[:, q0 + 512:q0 + 1024],
                                         start=True, stop=True)
                        nc.scalar.activation(
                            sigT[:, c, p * 1024:(p + 1) * 1024],
                            ps[:, :, :], AF.Sigmoid,
                            bias=bse[:, c:c + 1], scale=1.0,
                        )

                # ---- S finalize: harvest + transpose to [rows, DE] ----
                if do_tr:
                    with tc.tile_pool(name="sfin", bufs=1) as sf:
                        stT_sb = sf.tile([128, 4, SROWS], BF16,
                                         name="stT_sb")
                        nc.vector.tensor_scalar(
                            stT_sb[:, :, 0:2], Tacc[:], tmask[:], None,
                            op0=OP.mult)
                        nc.vector.tensor_copy(stT_sb[:, :, 2:SROWS],
                                              stT_ps[:, :, 2:SROWS])
                        with tc.tile_pool(name="strp", bufs=1,
                                          space="PSUM") as sp2:
                            s_trp = sp2.tile([32, 4, 128], BF16,
                                             name="s_trp")
                            for c in range(4):
                                nc.tensor.transpose(
                                    s_trp[0:SROWS, c, :], stT_sb[:, c, :],
                                    ident[:],
                                )
                            nc.vector.tensor_copy(
                                S_sb[0:SROWS, :], s_trp[0:SROWS, :, :])

            # ---- Phase 3: per own block: tri + offsets (same PSUM),
            # gate (DVE/Pool), query-major projection, copy+DMA out. ----
            with (
                tc.tile_pool(name="aggp", bufs=3, space="PSUM") as ap,
                tc.tile_pool(name="prjp", bufs=2, space="PSUM") as pp,
                tc.tile_pool(name="otp", bufs=3) as op_,
            ):
                prj = None
                for k in (range(OBLK) if do_p3 else ()):
                    j = OBLK + k
                    agg = ap.tile([128, 4, 128], F32, name="agg")
                    for c in range(4):
                        nc.tensor.matmul(
                            agg[:, c, :], H[c][:, j, :],
                            utri[:], start=(c == 0), stop=False,
                        )
                    if mode == "causal":
                        for c in range(4):
                            nc.tensor.matmul(
                                agg[:, c, :],
                                S_sb[0:2 + k, c * 128:(c + 1) * 128],
                                onesk[0:2 + k, :], start=False,
                                stop=(c == 3),
                            )
                    else:
                        for c in range(4):
                            nc.tensor.matmul(
                                agg[:, c, :],
                                S_sb[0:SROWS, c * 128:(c + 1) * 128],
                                onesk[0:SROWS, :], start=False, stop=False,
                            )
                            nc.tensor.matmul(
                                agg[:, c, :],
                                S_sb[2 + k:3 + k, c * 128:(c + 1) * 128],
                                negk[0:1, :], start=False, stop=(c == 3),
                            )
                    oT = op_.tile([128, 4, 128], BF16, name="oT")
                    # GPSIMD cannot read PSUM -> gates live on DVE.
                    nc.vector.tensor_tensor(
                        oT[:], agg[:],
                        sigT[:, :, k * 128:(k + 1) * 128], op=OP.mult,
                    )
                    if k % 4 == 0:
                        prj = pp.tile([128, 4, 128], F32, name="prj")
                    for c in range(4):
                        nc.tensor.matmul(
                            prj[:, k % 4, :], oT[:, c, :], wagT[:, c, :],
                            start=(k % 4 == 0 and c == 0),
                            stop=(k % 4 == 3 and c == 3),
                        )
                    if k % 4 == 3:
                        g = k // 4
                        nc.scalar.activation(
                            outQ[:, 4 * g:4 * g + 4, :], prj[:],
                            AF.Identity)
                        nc.sync.dma_start(
                            out_d[:, 4 * g:4 * g + 4, :],
                            outQ[:, 4 * g:4 * g + 4, :])

            if not do_p3:
                nc.sync.dma_start(out_d[:], outQ[:])
            if serialize:
                # Chain each iteration's output into the next iteration's
                # input loads so loop iterations measure full latency.
                for ch in range(8):
                    nc.sync.dma_start(
                        xqT[:, ch * 512:ch * 512 + 8], outQ[:, 0, 0:8])


def classify_mask(mask):
    mask = np.asarray(mask)
    m0 = np.asarray(mask[0], dtype=np.float32)
    for k in range(1, mask.shape[0]):
        if not np.array_equal(np.asarray(mask[k], dtype=np.float32), m0):
            return None
    if np.array_equal(m0, np.tril(np.ones((N, N), np.float32))):
        return "causal"
    if np.array_equal(m0, np.ones((N, N), np.float32)):
        return "ones"
    return None


def _bf16(a):
    import ml_dtypes
    return np.asarray(a, dtype=np.float32).astype(ml_dtypes.bfloat16)


def make_in_maps(xq, W_se, b_se, W_po, b_po, W_ag, b_ag, mode):
    f = lambda a: np.ascontiguousarray(np.asarray(a, dtype=np.float32))
    xq, W_se, b_se = f(xq), f(W_se), f(b_se)
    W_po, b_po, W_ag, b_ag = f(W_po), f(b_po), f(W_ag), f(b_ag)

    common = dict(
        wpoT=_bf16(W_po.T),
        wseT=_bf16(W_se.T),
        wagT=_bf16(W_ag.T.reshape(4, 128, 128).transpose(1, 0, 2)),
        bpoP=f(b_po.reshape(4, 128).T),
        bse=f(b_se.reshape(4, 128).T),
        utri=_bf16(
            np.triu(np.ones((128, 128), np.float32))
            if mode == "causal"
            else np.ones((128, 128), np.float32)
        ),
        onesk=_bf16(np.ones((32, 128), np.float32)),
        negk=_bf16(-np.ones((32, 128), np.float32)),
        ident=_bf16(np.eye(128, dtype=np.float32)),
    )

    in_maps = []
    for core in range(8):
        b, q = divmod(core, 2)
        if q == 1:
            xqp = xq[b]
        else:
            xqp = np.concatenate([xq[b, HALF:], xq[b, :HALF]], axis=0)
        # other-half mask: q=1 cores include the other (first) half total
        # in their offsets; q=0 cores' other half is causally after them.
        # ones mode: both halves always included.
        t = 1.0 if (q == 1 or mode == "ones") else 0.0
        onzT = np.zeros((128, 8), np.float32)
        onzT[:, 0] = 1.0
        onzT[:, 3] = 1.0
        onzT[:, 4] = t
        onzT[:, 7] = t
        in_maps.append(dict(
            common, xqT=_bf16(xqp.T), onzT=_bf16(onzT),
            tmask=np.full((128, 1), t, np.float32),
        ))
    return in_maps


def gather(results, mode):
    out = np.empty((B, N, DIM), np.float32)
    if mode == "causal":
        cnts = {
            1: (HALF + np.arange(HALF) + 1).astype(np.float64),
            0: (np.arange(HALF) + 1).astype(np.float64),
        }
    else:
        cnts = {1: np.full(HALF, float(N), np.float64),
                0: np.full(HALF, float(N), np.float64)}
    for core in range(8):
        b, q = divmod(core, 2)
        o = np.asarray(results[core]["outQ"]).astype(np.float32)
        o = o.transpose(1, 0, 2).reshape(HALF, DIM)
        invc = (1.0 / (1e-7 + cnts[q])).astype(np.float32)
        out[b, q * HALF:(q + 1) * HALF, :] = o * invc[:, None]
    return out


def _fallback(xq, mask, W_se, b_se, W_po, b_po, W_ag, b_ag):
    os.environ.setdefault("JAX_PLATFORMS", "cpu")
    import jax
    import jax.numpy as jnp

    with jax.default_device(jax.devices("cpu")[0]):
        s = jnp.asarray(xq) @ jnp.asarray(W_se).T + jnp.asarray(b_se)
        h = jnp.asarray(xq) @ jnp.asarray(W_po).T + jnp.asarray(b_po)
        g = jax.nn.gelu(h, approximate=False)
        h1, h2 = jnp.split(g, 2, axis=-1)
        h = jnp.concatenate([h1, h2 * h1], axis=-1)
        agg = jnp.einsum("bnd,bmn->bmd", h, jnp.asarray(mask))
        agg = agg / (1e-7 + jnp.sum(jnp.asarray(mask), axis=2, keepdims=True))
        o = jax.nn.sigmoid(s) * agg
        return np.asarray(o @ jnp.asarray(W_ag).T + jnp.asarray(b_ag))


def kernel(xq, mask, W_se, b_se, W_po, b_po, W_ag, b_ag):
    mode = classify_mask(mask)
    if mode is None:
        return _fallback(xq, mask, W_se, b_se, W_po, b_po, W_ag, b_ag)
    in_maps = make_in_maps(xq, W_se, b_se, W_po, b_po, W_ag, b_ag, mode)
    nc = build_nc(mode)
    res = run_bass_kernel_spmd(nc, in_maps, list(range(8)))
    out = gather(res.results, mode)
    return out + np.asarray(b_ag, np.float32)

